# revision 31
# baseline (speedup 1.0000x reference)
"""Trainium2 Bass kernel for nn_Barrier_Net (DeepSet MLPs + closest-barrier APF).

Sharding: pure data-parallel over the batch axis across 8 NeuronCores
(16384 rows/core); MLP parameters replicated.

Per-core dataflow:
  - x is supplied twice: fp32 row-major (APF path) and fp16 padded [R, 112]
    (MLP path). Per 2048-row chunk, one hardware xbar DMA-transpose loads
    the fp16 copy feature-major as xT [112, 2048] directly into SBUF.
  - phi branches: per PAIR of set elements, K-padded lhsT [101,128] matmuls
    (L1), block-diag [128,128] (L2), stacked [128,16] accumulating (L3).
    Relu+bias fused into the PSUM->SBUF copies (h1 on ACT, h2/rh1 on DVE,
    bias read from engine-local constant copies).
  - rho/psi: small matmuls; linear biases folded into downstream weights.
    psi layer-2 runs per-128-row-subtile with the activations as lhsT so
    the result lands batch-major.
  - APF (dists/argmin/gather) runs batch-major entirely on GPSIMD with
    tree-reductions; ties resolve to the first index (match on index).
  - Epilogue (sqrt/reciprocal/tanh) batched over the whole core.

The walrus build on this stack accepts only ONE sync wait per engine
instruction; `_legalize_waits` hoists extras onto injected same-engine
EventSemaphore carriers after Tile scheduling.
"""

import sys

sys.path.insert(0, "/opt/trn_rl_repo")

from contextlib import ExitStack

import numpy as np

import concourse.bass as bass
import concourse.mybir as mybir
import concourse.tile as tile
from concourse.bass_utils import run_bass_kernel_spmd

F32 = mybir.dt.float32
F16 = mybir.dt.float16
AF = mybir.ActivationFunctionType
OP = mybir.AluOpType
AX = mybir.AxisListType

B = 131072
NCORES = 8
B_GAMMA = 0.01

# ---------------------------------------------------------------------------
# numpy-side constant preparation
# ---------------------------------------------------------------------------


def _np(t):
    return np.asarray(t, dtype=np.float32)


def _prep_consts(phi_n, rho_n, phi_o, rho_o, psi):
    (W1n, b1n), (W2n, b2n), (W3n, b3n) = [(_np(w), _np(b)) for w, b in phi_n]
    (Wr1n, br1n), (Wr2n, br2n) = [(_np(w), _np(b)) for w, b in rho_n]
    (W1o, b1o), (W2o, b2o), (W3o, b3o) = [(_np(w), _np(b)) for w, b in phi_o]
    (Wr1o, br1o), (Wr2o, br2o) = [(_np(w), _np(b)) for w, b in rho_o]
    (Wp1, bp1), (Wp2, bp2) = [(_np(w), _np(b)) for w, b in psi]

    hf = np.float16
    C = {}

    # L1: 16 padded lhsT tiles [101, 128]; tile i pairs elems (2i, 2i+1).
    w_l1 = np.zeros((101, 16, 128), np.float32)
    for i in range(8):
        j0, j1 = 2 * i, 2 * i + 1
        w_l1[5 + 4 * j0 : 9 + 4 * j0, i, 0:64] = W1n
        w_l1[5 + 4 * j1 : 9 + 4 * j1, i, 64:128] = W1n
        w_l1[69 + 2 * j0 : 71 + 2 * j0, 8 + i, 0:64] = W1o
        w_l1[69 + 2 * j1 : 71 + 2 * j1, 8 + i, 64:128] = W1o
    C["w_l1"] = w_l1.astype(hf)

    w_l2 = np.zeros((128, 2, 128), np.float32)
    w_l2[0:64, 0, 0:64] = W2n
    w_l2[64:128, 0, 64:128] = W2n
    w_l2[0:64, 1, 0:64] = W2o
    w_l2[64:128, 1, 64:128] = W2o
    C["w_l2"] = w_l2.astype(hf)

    w_l3 = np.zeros((128, 2, 16), np.float32)
    w_l3[0:64, 0, :] = W3n
    w_l3[64:128, 0, :] = W3n
    w_l3[0:64, 1, :] = W3o
    w_l3[64:128, 1, :] = W3o
    C["w_l3"] = w_l3.astype(hf)

    # rho L1 [16, 2, 64]; 16*b3 folded into the relu bias.
    w_rho1 = np.zeros((16, 2, 64), np.float32)
    w_rho1[:, 0, :] = Wr1n
    w_rho1[:, 1, :] = Wr1o
    C["w_rho1"] = w_rho1.astype(hf)
    br1n_f = br1n + 16.0 * (b3n @ Wr1n)
    br1o_f = br1o + 16.0 * (b3o @ Wr1o)

    # rho L2: n-branch padded to [64, 64] (cols 16-63 zero) so the psi psum
    # staging tile rows 0-63 are fully written; o-branch writes rows 64-79.
    w_rho2n = np.zeros((64, 64), np.float32)
    w_rho2n[:, 0:16] = Wr2n
    C["w_rho2n"] = w_rho2n.astype(hf)
    C["w_rho2o"] = Wr2o.astype(hf)

    # psi L1 split: g-part reads xT (rows 1-4); rho parts read the staged
    # psi_in tile (ne at 0-15, ob at 64-79). rho L2 biases folded in.
    w_psi1g = np.zeros((101, 64), np.float32)
    w_psi1g[1:5, :] = Wp1[32:36, :]
    C["w_psi1g"] = w_psi1g.astype(hf)
    w_psi1 = np.zeros((80, 64), np.float32)
    w_psi1[0:16, :] = Wp1[0:16, :]
    w_psi1[64:80, :] = Wp1[16:32, :]
    C["w_psi1"] = w_psi1.astype(hf)
    bp1_f = bp1 + br2n @ Wp1[0:16, :] + br2o @ Wp1[16:32, :]

    C["w_psi2"] = Wp2.astype(hf)

    bias = np.zeros((128, 10), np.float32)
    bias[0:64, 0] = b1n
    bias[64:128, 0] = b1n
    bias[0:64, 1] = b2n
    bias[64:128, 1] = b2n
    bias[0:64, 2] = b1o
    bias[64:128, 2] = b1o
    bias[0:64, 3] = b2o
    bias[64:128, 3] = b2o
    bias[0:64, 4] = br1n_f
    bias[0:64, 5] = br1o_f
    bias[0:64, 6] = bp1_f
    bias[:, 7] = bp2[0]
    bias[:, 8] = bp2[1]
    C["bias"] = bias

    iota_d = np.zeros((128, 32), np.float32)
    iota_d[:] = 32.0 - np.arange(32, dtype=np.float32)[None, :]
    C["iota_d"] = iota_d
    return C


# ---------------------------------------------------------------------------
# helpers
# ---------------------------------------------------------------------------


def _bc_mid(ap, mid):
    return bass.AP(tensor=ap.tensor, offset=ap.offset,
                   ap=[ap.ap[0], [0, mid], ap.ap[1]])


def _bc_inner(ap, inner):
    return bass.AP(tensor=ap.tensor, offset=ap.offset,
                   ap=list(ap.ap) + [[0, inner]])


def _tt(eng, out, in0, in1, op):
    """tensor_tensor via InstTensorScalarPtr (TT has only 1 HW wait slot)."""
    eng.scalar_tensor_tensor(out=out, in0=in0, scalar=0.0, in1=in1,
                             op0=OP.bypass, op1=op)


_NO_SPLIT = {
    "InstUnconditionalBranch", "InstCall", "InstRegisterMove",
    "InstISA", "InstEventSemaphore",
}


def _legalize_waits(nc):
    """Split multi-wait instructions: walrus here accepts one wait each."""
    def fix_block(bb):
        out = []
        for inst in bb.instructions:
            si = inst.sync_info
            waits = list(si.on_wait) if si else []
            if type(inst).__name__ not in _NO_SPLIT and len(waits) > 1:
                for k, w in enumerate(waits[:-1]):
                    ev = mybir.InstEventSemaphore(
                        name=f"{inst.name}-waitsplit{k}", ins=[], outs=[])
                    ev.engine = inst.engine
                    ev.sync_info = mybir.SyncInfo(on_wait=[w], on_update=[])
                    out.append(ev)
                inst.sync_info = mybir.SyncInfo(
                    on_wait=[waits[-1]], on_update=list(si.on_update))
            out.append(inst)
        bb.instructions = out

    def walk(bbs):
        for bb in bbs:
            fix_block(bb)
            walk(getattr(bb, "blocks", []) or [])

    walk(nc.m.functions[0].blocks)


# ---------------------------------------------------------------------------
# bass program
# ---------------------------------------------------------------------------


def _build(rows):
    nchunks = rows // 2048
    nc = bass.Bass(trn_type="TRN2")

    x_in = nc.dram_tensor("x_in", (rows, 101), F32, kind="ExternalInput")
    x16_in = nc.dram_tensor("x16_in", (rows, 128), F16, kind="ExternalInput")
    out = nc.dram_tensor("out", (rows, 2), F32, kind="ExternalOutput")

    d_wl1 = nc.dram_tensor("w_l1", (101, 16, 128), F16, kind="ExternalInput")
    d_wl2 = nc.dram_tensor("w_l2", (128, 2, 128), F16, kind="ExternalInput")
    d_wl3 = nc.dram_tensor("w_l3", (128, 2, 16), F16, kind="ExternalInput")
    d_wr1 = nc.dram_tensor("w_rho1", (16, 2, 64), F16, kind="ExternalInput")
    d_wr2n = nc.dram_tensor("w_rho2n", (64, 64), F16, kind="ExternalInput")
    d_wr2o = nc.dram_tensor("w_rho2o", (64, 16), F16, kind="ExternalInput")
    d_wp1g = nc.dram_tensor("w_psi1g", (101, 64), F16, kind="ExternalInput")
    d_wp1 = nc.dram_tensor("w_psi1", (80, 64), F16, kind="ExternalInput")
    d_wp2 = nc.dram_tensor("w_psi2", (64, 2), F16, kind="ExternalInput")
    d_bias = nc.dram_tensor("bias", (128, 10), F32, kind="ExternalInput")
    d_iota = nc.dram_tensor("iota_d", (128, 32), F32, kind="ExternalInput")

    with tile.TileContext(nc) as tc, ExitStack() as ctx:
        singles = ctx.enter_context(tc.tile_pool(name="singles", bufs=1))
        xpool = ctx.enter_context(tc.tile_pool(name="xpool", bufs=2))
        ps_big = ctx.enter_context(tc.tile_pool(name="ps_big", bufs=4, space="PSUM"))
        ps_acc = ctx.enter_context(tc.tile_pool(name="ps_acc", bufs=3, space="PSUM"))
        ps_z = ctx.enter_context(tc.tile_pool(name="ps_z", bufs=1, space="PSUM"))

        def load(name, shape, dt, dram):
            t = singles.tile(shape, dt, tag=name, name=name + "_sb")
            nc.sync.dma_start(out=t, in_=dram[tuple(slice(None) for _ in shape)])
            return t

        w_l1 = load("w_l1", [101, 16, 128], F16, d_wl1)
        w_l2 = load("w_l2", [128, 2, 128], F16, d_wl2)
        w_l3 = load("w_l3", [128, 2, 16], F16, d_wl3)
        w_rho1 = load("w_rho1", [16, 2, 64], F16, d_wr1)
        w_rho2n = load("w_rho2n", [64, 64], F16, d_wr2n)
        w_rho2o = load("w_rho2o", [64, 16], F16, d_wr2o)
        w_psi1g = load("w_psi1g", [101, 64], F16, d_wp1g)
        w_psi1 = load("w_psi1", [80, 64], F16, d_wp1)
        w_psi2 = load("w_psi2", [64, 2], F16, d_wp2)
        bias = load("bias", [128, 10], F32, d_bias)
        iota_d = load("iota_d", [128, 32], F32, d_iota)

        psi_in = singles.tile([80, 512], F16, tag="psi_in")
        nc.vector.memset(psi_in, 0.0)

        # Warm-up: touch weights on PE; engine-local bias/iota copies so no
        # steady-state instruction spends a wait slot on constant DMAs.
        for wt in [w_l1[:, 0, :], w_l2[:, 0, :], w_l3[:, 0, :],
                   w_rho1[:, 0, :], w_rho2n, w_rho2o, w_psi1g, w_psi1,
                   w_psi2]:
            nc.tensor.ldweights(weights=wt)
        bias_a = singles.tile([128, 10], F32, tag="bias_a")
        nc.scalar.copy(bias_a, bias)
        bias_v = singles.tile([128, 10], F32, tag="bias_v")
        nc.vector.tensor_copy(out=bias_v, in_=bias)
        iota_p = singles.tile([128, 32], F32, tag="iota_p")
        nc.gpsimd.tensor_copy(out=iota_p, in_=iota_d)

        # static ping-pong work tiles
        xts = [singles.tile([128, 2048], F16, tag=f"xt{i}", name=f"xt{i}")
               for i in range(2)]
        h1s = [singles.tile([128, 2, 512], F16, tag=f"h1{i}", name=f"h1{i}")
               for i in range(3)]
        h2s = [singles.tile([128, 2, 512], F16, tag=f"h2{i}", name=f"h2{i}")
               for i in range(3)]
        rh1s = [singles.tile([128, 2, 512], F16, tag=f"rh1{i}", name=f"rh1{i}")
                for i in range(2)]
        rins = [[singles.tile([16, 512], F16, tag=f"rin{b}{i}",
                              name=f"rin{b}{i}") for i in range(2)]
                for b in range(2)]
        psihs = [singles.tile([64, 512], F16, tag=f"psih{i}", name=f"psih{i}")
                 for i in range(2)]
        d2 = singles.tile([128, 16, 32], F32, tag="d2")
        t32 = singles.tile([128, 16, 32], F32, tag="t32")
        tr = singles.tile([128, 16, 32], F32, tag="tr")
        vx = singles.tile([128, 16, 16], F32, tag="vx")
        vy = singles.tile([128, 16, 16], F32, tag="vy")
        eq = singles.tile([128, 16, 32], F32, tag="eq")
        oh = singles.tile([128, 16, 32], F32, tag="oh")

        sx_all = singles.tile([128, nchunks, 16], F32, tag="sx")
        sy_all = singles.tile([128, nchunks, 16], F32, tag="sy")
        dg_all = singles.tile([128, nchunks, 16], F32, tag="dg")
        q_all = singles.tile([128, nchunks, 16], F32, tag="q")
        zx_all = singles.tile([128, nchunks, 16], F32, tag="zx")
        zy_all = singles.tile([128, nchunks, 16], F32, tag="zy")
        out_xy = singles.tile([128, nchunks, 16, 2], F32, tag="oxy")

        x_view = x_in.rearrange("(p c s) f -> p c s f", p=128, c=nchunks)
        x16_view = x16_in.rearrange("(c r) f -> c r f", c=nchunks)
        iota_b = _bc_mid(iota_p[:, :], 16)

        def relu_copy(dst, src, bias_ap, eng):
            if eng is nc.scalar:
                nc.scalar.activation(dst, src, AF.Relu, bias=bias_ap, scale=1.0)
            else:
                nc.vector.tensor_scalar(
                    out=dst, in0=src, scalar1=bias_ap, scalar2=0.0,
                    op0=OP.add, op1=OP.max,
                )

        def tree_reduce(g_, buf, n0, op, out_ap=None):
            """In-place halving tree over the innermost dim of buf."""
            n = n0 // 2
            while n >= 1:
                dst = buf[:, :, 0:n] if (n > 1 or out_ap is None) else out_ap
                _tt(g_, dst, buf[:, :, 0:n], buf[:, :, n:2 * n], op)
                n //= 2

        def trace_branch(ch, g, xt_c, z_ps):
            xt = xt_c[0:101, g * 512:(g + 1) * 512]
            acc = ps_acc.tile([80, 512], F32, tag="acc", name="acc")
            for br in range(2):
                for p4 in range(4):
                    h1 = h1s[p4 % 3]
                    for h in range(2):
                        h1ps = ps_big.tile([128, 512], F32, tag="big",
                                           name="h1ps")
                        nc.tensor.matmul(
                            h1ps, w_l1[:, br * 8 + p4 * 2 + h, :], xt,
                            start=True, stop=True,
                        )
                        relu_copy(h1[:, h, :], h1ps,
                                  bias_a[:, 2 * br:2 * br + 1], nc.scalar)
                    h2 = h2s[p4 % 3]
                    for h in range(2):
                        h2ps = ps_big.tile([128, 512], F32, tag="big",
                                           name="h2ps")
                        nc.tensor.matmul(
                            h2ps, w_l2[:, br, :], h1[:, h, :],
                            start=True, stop=True,
                        )
                        relu_copy(h2[:, h, :], h2ps,
                                  bias_v[:, 2 * br + 1:2 * br + 2], nc.vector)
                    for h in range(2):
                        nc.tensor.matmul(
                            acc[32 * br:32 * br + 16, :],
                            w_l3[:, br, :], h2[:, h, :],
                            start=(p4 == 0 and h == 0),
                            stop=(p4 == 3 and h == 1),
                        )
            return acc

        def trace_tail(ch, g, xt_c, z_ps, acc):
            xt = xt_c[0:101, g * 512:(g + 1) * 512]
            rin_n = rins[0][g % 2]
            nc.scalar.copy(rin_n, acc[0:16, :])
            rin_o = rins[1][g % 2]
            nc.scalar.copy(rin_o, acc[32:48, :])

            rh1 = rh1s[g % 2]
            r1ps = ps_big.tile([128, 512], F32, tag="big", name="r1ps")
            nc.tensor.matmul(r1ps[0:64, :], w_rho1[:, 0, :], rin_n,
                             start=True, stop=True)
            relu_copy(rh1[0:64, 0, :], r1ps[0:64, :], bias_v[0:64, 4:5],
                      nc.vector)
            r1ps2 = ps_big.tile([128, 512], F32, tag="big", name="r1ps2")
            nc.tensor.matmul(r1ps2[0:64, :], w_rho1[:, 1, :], rin_o,
                             start=True, stop=True)
            relu_copy(rh1[0:64, 1, :], r1ps2[0:64, :], bias_v[0:64, 5:6],
                      nc.vector)

            psi_ps = ps_acc.tile([80, 512], F32, tag="acc", name="psi_ps")
            nc.tensor.matmul(psi_ps[0:64, :], w_rho2n, rh1[0:64, 0, :],
                             start=True, stop=True)
            nc.tensor.matmul(psi_ps[64:80, :], w_rho2o, rh1[0:64, 1, :],
                             start=True, stop=True)
            nc.scalar.copy(psi_in[0:80, :], psi_ps[0:80, :])

            ph_ps = ps_big.tile([128, 512], F32, tag="big", name="ph_ps")
            nc.tensor.matmul(ph_ps[0:64, :], w_psi1g, xt,
                             start=True, stop=False)
            nc.tensor.matmul(ph_ps[0:64, :], w_psi1, psi_in,
                             start=False, stop=True)
            psi_h = psihs[g % 2]
            relu_copy(psi_h, ph_ps[0:64, :], bias_a[0:64, 6:7], nc.scalar)

            for sl in range(4):
                nc.tensor.matmul(
                    z_ps[:, 4 * g + sl, :],
                    psi_h[:, sl * 128:(sl + 1) * 128], w_psi2,
                    start=True, stop=True,
                )

        def trace_apf(ch, x_ch, z_ps):
            nc.vector.tensor_copy(out=zx_all[:, ch, :], in_=z_ps[:, :, 0])
            nc.vector.tensor_copy(out=zy_all[:, ch, :], in_=z_ps[:, :, 1])

            nall = x_ch[:, :, 5:69].rearrange("p s (j c) -> p s j c", c=4)
            nx = nall[:, :, :, 0]
            ny = nall[:, :, :, 1]
            oall = x_ch[:, :, 69:101].rearrange("p s (j c) -> p s j c", c=2)
            ox = oall[:, :, :, 0]
            oy = oall[:, :, :, 1]

            g_ = nc.vector
            _tt(g_, d2[:, :, 0:16], nx, nx, OP.mult)
            _tt(g_, t32[:, :, 0:16], ny, ny, OP.mult)
            _tt(g_, d2[:, :, 0:16], d2[:, :, 0:16], t32[:, :, 0:16], OP.add)
            g_.tensor_scalar(out=t32[:, :, 0:16], in0=ox, scalar1=-0.5,
                             scalar2=0.5, op0=OP.max, op1=OP.min)
            _tt(g_, vx, ox, t32[:, :, 0:16], OP.subtract)
            g_.tensor_scalar(out=t32[:, :, 0:16], in0=oy, scalar1=-0.5,
                             scalar2=0.5, op0=OP.max, op1=OP.min)
            _tt(g_, vy, oy, t32[:, :, 0:16], OP.subtract)
            _tt(g_, d2[:, :, 16:32], vx, vx, OP.mult)
            _tt(g_, t32[:, :, 16:32], vy, vy, OP.mult)
            _tt(g_, d2[:, :, 16:32], d2[:, :, 16:32], t32[:, :, 16:32],
                OP.add)

            g_.tensor_reduce(out=tr[:, :, 0], in_=d2, axis=AX.X, op=OP.min)
            _tt(g_, eq, d2, _bc_inner(tr[:, :, 0], 32), OP.is_equal)
            _tt(g_, eq, eq, iota_b, OP.mult)
            g_.tensor_reduce(out=tr[:, :, 1], in_=eq, axis=AX.X, op=OP.max)
            _tt(g_, oh, iota_b, _bc_inner(tr[:, :, 1], 32), OP.is_equal)

            _tt(g_, t32[:, :, 0:16], oh[:, :, 0:16], nx, OP.mult)
            _tt(g_, t32[:, :, 16:32], oh[:, :, 16:32], vx, OP.mult)
            g_.tensor_reduce(out=sx_all[:, ch, :], in_=t32, axis=AX.X,
                             op=OP.add)
            _tt(g_, t32[:, :, 0:16], oh[:, :, 0:16], ny, OP.mult)
            _tt(g_, t32[:, :, 16:32], oh[:, :, 16:32], vy, OP.mult)
            g_.tensor_reduce(out=sy_all[:, ch, :], in_=t32, axis=AX.X,
                             op=OP.add)

            g_.tensor_scalar(out=dg_all[:, ch, :], in0=tr[:, :, 1],
                             scalar1=16.0, scalar2=0.15, op0=OP.is_gt,
                             op1=OP.mult)
            _tt(g_, q_all[:, ch, :], sx_all[:, ch, :], sx_all[:, ch, :],
                OP.mult)
            _tt(g_, t32[:, 0, 0:16], sy_all[:, ch, :], sy_all[:, ch, :],
                OP.mult)
            _tt(g_, q_all[:, ch, :], q_all[:, ch, :], t32[:, 0, 0:16], OP.add)

        # software pipeline: tail of stage k runs while stage k+1's branch
        # matmul stream keeps every queue fed.
        state = {}
        pending = None  # (ch, g, xt_c, z_ps, acc)
        for ch in range(nchunks):
            x_ch = xpool.tile([128, 16, 101], F32, tag="x", name="x_ch")
            nc.sync.dma_start(out=x_ch, in_=x_view[:, ch, :, :])
            xt_c = xts[ch % 2]
            nc.sync.dma_start_transpose(out=xt_c, in_=x16_view[ch, :, :])
            z_ps = ps_z.tile([128, 16, 2], F32, tag="z", name="z_ps")
            state[ch] = (x_ch, z_ps)
            for g in range(4):
                acc = trace_branch(ch, g, xt_c, z_ps)
                if pending is not None:
                    trace_tail(*pending)
                    pch, pg = pending[0], pending[1]
                    if pg == 3:
                        trace_apf(pch, *state.pop(pch))
                pending = (ch, g, xt_c, z_ps, acc)
        trace_tail(*pending)
        trace_apf(pending[0], *state.pop(pending[0]))

        # ---- epilogue ----
        normp = singles.tile([128, nchunks, 16], F32, tag="normp")
        nc.scalar.sqrt(normp, q_all)
        hb = singles.tile([128, nchunks, 16], F32, tag="hb")
        nc.vector.scalar_tensor_tensor(
            out=hb, in0=normp, scalar=0.15, in1=dg_all,
            op0=OP.subtract, op1=OP.subtract,
        )
        den = singles.tile([128, nchunks, 16], F32, tag="den")
        _tt(nc.vector, den, normp, hb, OP.mult)
        rden = singles.tile([128, nchunks, 16], F32, tag="rden")
        nc.vector.reciprocal(out=rden, in_=den)
        rsc = singles.tile([128, nchunks, 16], F32, tag="rsc")
        nc.vector.tensor_scalar_mul(out=rsc, in0=rden, scalar1=-B_GAMMA)
        bx = singles.tile([128, nchunks, 16], F32, tag="bx")
        _tt(nc.vector, bx, sx_all, rsc, OP.mult)
        by = singles.tile([128, nchunks, 16], F32, tag="by")
        _tt(nc.vector, by, sy_all, rsc, OP.mult)

        tx = singles.tile([128, nchunks, 16], F32, tag="tx")
        nc.scalar.activation(tx, zx_all, AF.Tanh, bias=bias_a[:, 7:8], scale=1.0)
        ty = singles.tile([128, nchunks, 16], F32, tag="ty")
        nc.scalar.activation(ty, zy_all, AF.Tanh, bias=bias_a[:, 8:9], scale=1.0)
        ex = singles.tile([128, nchunks, 16], F32, tag="ex")
        nc.vector.scalar_tensor_tensor(out=ex, in0=tx, scalar=2.0, in1=bx,
                                       op0=OP.mult, op1=OP.add)
        ey = singles.tile([128, nchunks, 16], F32, tag="ey")
        nc.vector.scalar_tensor_tensor(out=ey, in0=ty, scalar=2.0, in1=by,
                                       op0=OP.mult, op1=OP.add)
        fx = singles.tile([128, nchunks, 16], F32, tag="fx")
        nc.scalar.activation(fx, ex, AF.Tanh, bias=0.0, scale=1.0)
        fy = singles.tile([128, nchunks, 16], F32, tag="fy")
        nc.scalar.activation(fy, ey, AF.Tanh, bias=0.0, scale=1.0)
        nc.vector.tensor_scalar_mul(out=out_xy[:, :, :, 0], in0=fx, scalar1=2.0)
        nc.vector.tensor_scalar_mul(out=out_xy[:, :, :, 1], in0=fy, scalar1=2.0)

        out_view = out.rearrange("(p c s) t -> p c s t", p=128, c=nchunks)
        nc.sync.dma_start(out=out_view, in_=out_xy)

    _legalize_waits(nc)
    return nc


# ---------------------------------------------------------------------------
# entry point
# ---------------------------------------------------------------------------

_CACHE = {}


def _get_program(rows):
    if rows not in _CACHE:
        _CACHE[rows] = _build(rows)
    return _CACHE[rows]


def _x16_of(x, rows):
    """fp16 copy of x, 128-col padded, rows permuted per core so that the
    xbar transpose's column order (linear within a 2048-chunk) matches the
    batch-major row mapping row = p*(nchunks*16) + ch*16 + s."""
    nchunks = rows // 2048
    x16 = np.zeros((x.shape[0], 128), np.float16)
    x16[:, 0:101] = x
    p = np.arange(128)
    ch = np.arange(nchunks)
    s = np.arange(16)
    # destination index ch*2048 + s*128 + p  <-  source index p*(nchunks*16) + ch*16 + s
    src_idx = (p[None, None, :] * (nchunks * 16) + ch[:, None, None] * 16
               + s[None, :, None])  # [ch, s, p]
    perm = src_idx.reshape(-1)
    ncores = x.shape[0] // rows
    outp = np.empty_like(x16)
    for c in range(ncores):
        outp[c * rows:(c + 1) * rows] = x16[c * rows:(c + 1) * rows][perm]
    return outp


def kernel(x, phi_n, rho_n, phi_o, rho_o, psi, _rows_per_core=None,
           _core_ids=None, _trace=False):
    x = np.ascontiguousarray(np.asarray(x, dtype=np.float32))
    C = _prep_consts(phi_n, rho_n, phi_o, rho_o, psi)

    rows = _rows_per_core or (x.shape[0] // NCORES)
    core_ids = _core_ids if _core_ids is not None else list(range(NCORES))
    ncores = len(core_ids)

    nc = _get_program(rows)
    x16 = _x16_of(x, rows)

    const_map = {k: C[k] for k in
                 ["w_l1", "w_l2", "w_l3", "w_rho1", "w_rho2n", "w_rho2o",
                  "w_psi1g", "w_psi1", "w_psi2", "bias", "iota_d"]}
    in_maps = []
    for i in range(ncores):
        m = dict(const_map)
        m["x_in"] = x[i * rows: (i + 1) * rows]
        m["x16_in"] = x16[i * rows: (i + 1) * rows]
        in_maps.append(m)

    res = run_bass_kernel_spmd(nc, in_maps, core_ids=core_ids, trace=_trace)
    outp = np.concatenate([r["out"] for r in res.results], axis=0)
    if _trace:
        return outp, res
    return outp


# revision 34
# speedup vs baseline: 134.6699x; 134.6699x over previous
"""Trainium2 Bass kernel for nn_Barrier_Net (DeepSet MLPs + closest-barrier APF).

Sharding: pure data-parallel over the batch axis across 8 NeuronCores
(16384 rows/core); MLP parameters replicated.

Per-core dataflow:
  - x is supplied twice: fp32 row-major (APF path) and fp16 padded [R, 112]
    (MLP path). Per 2048-row chunk, one hardware xbar DMA-transpose loads
    the fp16 copy feature-major as xT [112, 2048] directly into SBUF.
  - phi branches: per PAIR of set elements, K-padded lhsT [101,128] matmuls
    (L1), block-diag [128,128] (L2), stacked [128,16] accumulating (L3).
    Relu+bias fused into the PSUM->SBUF copies (h1 on ACT, h2/rh1 on DVE,
    bias read from engine-local constant copies).
  - rho/psi: small matmuls; linear biases folded into downstream weights.
    psi layer-2 runs per-128-row-subtile with the activations as lhsT so
    the result lands batch-major.
  - APF (dists/argmin/gather) runs batch-major entirely on GPSIMD with
    tree-reductions; ties resolve to the first index (match on index).
  - Epilogue (sqrt/reciprocal/tanh) batched over the whole core.

The walrus build on this stack accepts only ONE sync wait per engine
instruction; `_legalize_waits` hoists extras onto injected same-engine
EventSemaphore carriers after Tile scheduling.
"""

import sys

sys.path.insert(0, "/opt/trn_rl_repo")

from contextlib import ExitStack

import numpy as np

import concourse.bass as bass
import concourse.mybir as mybir
import concourse.tile as tile
from concourse.bass_utils import run_bass_kernel_spmd

F32 = mybir.dt.float32
F16 = mybir.dt.float16
AF = mybir.ActivationFunctionType
OP = mybir.AluOpType
AX = mybir.AxisListType

B = 131072
NCORES = 8
B_GAMMA = 0.01

# ---------------------------------------------------------------------------
# numpy-side constant preparation
# ---------------------------------------------------------------------------


def _np(t):
    return np.asarray(t, dtype=np.float32)


def _prep_consts(phi_n, rho_n, phi_o, rho_o, psi):
    (W1n, b1n), (W2n, b2n), (W3n, b3n) = [(_np(w), _np(b)) for w, b in phi_n]
    (Wr1n, br1n), (Wr2n, br2n) = [(_np(w), _np(b)) for w, b in rho_n]
    (W1o, b1o), (W2o, b2o), (W3o, b3o) = [(_np(w), _np(b)) for w, b in phi_o]
    (Wr1o, br1o), (Wr2o, br2o) = [(_np(w), _np(b)) for w, b in rho_o]
    (Wp1, bp1), (Wp2, bp2) = [(_np(w), _np(b)) for w, b in psi]

    hf = np.float16
    C = {}

    # L1: 16 padded lhsT tiles [101, 128]; tile i pairs elems (2i, 2i+1).
    w_l1 = np.zeros((101, 16, 128), np.float32)
    for i in range(8):
        j0, j1 = 2 * i, 2 * i + 1
        w_l1[5 + 4 * j0 : 9 + 4 * j0, i, 0:64] = W1n
        w_l1[5 + 4 * j1 : 9 + 4 * j1, i, 64:128] = W1n
        w_l1[69 + 2 * j0 : 71 + 2 * j0, 8 + i, 0:64] = W1o
        w_l1[69 + 2 * j1 : 71 + 2 * j1, 8 + i, 64:128] = W1o
    C["w_l1"] = w_l1.astype(hf)

    w_l2 = np.zeros((128, 2, 128), np.float32)
    w_l2[0:64, 0, 0:64] = W2n
    w_l2[64:128, 0, 64:128] = W2n
    w_l2[0:64, 1, 0:64] = W2o
    w_l2[64:128, 1, 64:128] = W2o
    C["w_l2"] = w_l2.astype(hf)

    w_l3 = np.zeros((128, 2, 16), np.float32)
    w_l3[0:64, 0, :] = W3n
    w_l3[64:128, 0, :] = W3n
    w_l3[0:64, 1, :] = W3o
    w_l3[64:128, 1, :] = W3o
    C["w_l3"] = w_l3.astype(hf)

    # rho L1 [16, 2, 64]; 16*b3 folded into the relu bias.
    w_rho1 = np.zeros((16, 2, 64), np.float32)
    w_rho1[:, 0, :] = Wr1n
    w_rho1[:, 1, :] = Wr1o
    C["w_rho1"] = w_rho1.astype(hf)
    br1n_f = br1n + 16.0 * (b3n @ Wr1n)
    br1o_f = br1o + 16.0 * (b3o @ Wr1o)

    # rho L2: n-branch padded to [64, 64] (cols 16-63 zero) so the psi psum
    # staging tile rows 0-63 are fully written; o-branch writes rows 64-79.
    w_rho2n = np.zeros((64, 64), np.float32)
    w_rho2n[:, 0:16] = Wr2n
    C["w_rho2n"] = w_rho2n.astype(hf)
    C["w_rho2o"] = Wr2o.astype(hf)

    # psi L1 split: g-part reads xT (rows 1-4); rho parts read the staged
    # psi_in tile (ne at 0-15, ob at 64-79). rho L2 biases folded in.
    w_psi1g = np.zeros((101, 64), np.float32)
    w_psi1g[1:5, :] = Wp1[32:36, :]
    C["w_psi1g"] = w_psi1g.astype(hf)
    w_psi1 = np.zeros((80, 64), np.float32)
    w_psi1[0:16, :] = Wp1[0:16, :]
    w_psi1[64:80, :] = Wp1[16:32, :]
    C["w_psi1"] = w_psi1.astype(hf)
    bp1_f = bp1 + br2n @ Wp1[0:16, :] + br2o @ Wp1[16:32, :]

    C["w_psi2"] = Wp2.astype(hf)

    bias = np.zeros((128, 10), np.float32)
    bias[0:64, 0] = b1n
    bias[64:128, 0] = b1n
    bias[0:64, 1] = b2n
    bias[64:128, 1] = b2n
    bias[0:64, 2] = b1o
    bias[64:128, 2] = b1o
    bias[0:64, 3] = b2o
    bias[64:128, 3] = b2o
    bias[0:64, 4] = br1n_f
    bias[0:64, 5] = br1o_f
    bias[0:64, 6] = bp1_f
    bias[:, 7] = bp2[0]
    bias[:, 8] = bp2[1]
    C["bias"] = bias

    iota_d = np.zeros((128, 32), np.float32)
    iota_d[:] = 32.0 - np.arange(32, dtype=np.float32)[None, :]
    C["iota_d"] = iota_d
    return C


# ---------------------------------------------------------------------------
# helpers
# ---------------------------------------------------------------------------


def _bc_mid(ap, mid):
    return bass.AP(tensor=ap.tensor, offset=ap.offset,
                   ap=[ap.ap[0], [0, mid], ap.ap[1]])


def _bc_inner(ap, inner):
    return bass.AP(tensor=ap.tensor, offset=ap.offset,
                   ap=list(ap.ap) + [[0, inner]])


def _tt(eng, out, in0, in1, op):
    """tensor_tensor via InstTensorScalarPtr (TT has only 1 HW wait slot)."""
    eng.scalar_tensor_tensor(out=out, in0=in0, scalar=0.0, in1=in1,
                             op0=OP.bypass, op1=op)


_NO_SPLIT = {
    "InstUnconditionalBranch", "InstCall", "InstRegisterMove",
    "InstISA", "InstEventSemaphore",
}


def _legalize_waits(nc):
    """Split multi-wait instructions: walrus here accepts one wait each."""
    def fix_block(bb):
        out = []
        for inst in bb.instructions:
            si = inst.sync_info
            waits = list(si.on_wait) if si else []
            if type(inst).__name__ not in _NO_SPLIT and len(waits) > 1:
                for k, w in enumerate(waits[:-1]):
                    ev = mybir.InstEventSemaphore(
                        name=f"{inst.name}-waitsplit{k}", ins=[], outs=[])
                    ev.engine = inst.engine
                    ev.sync_info = mybir.SyncInfo(on_wait=[w], on_update=[])
                    out.append(ev)
                inst.sync_info = mybir.SyncInfo(
                    on_wait=[waits[-1]], on_update=list(si.on_update))
            out.append(inst)
        bb.instructions = out

    def walk(bbs):
        for bb in bbs:
            fix_block(bb)
            walk(getattr(bb, "blocks", []) or [])

    walk(nc.m.functions[0].blocks)


# ---------------------------------------------------------------------------
# bass program
# ---------------------------------------------------------------------------


def _build(rows):
    nchunks = rows // 2048
    nc = bass.Bass(trn_type="TRN2")

    x_in = nc.dram_tensor("x_in", (rows, 101), F32, kind="ExternalInput")
    x16_in = nc.dram_tensor("x16_in", (rows, 128), F16, kind="ExternalInput")
    out = nc.dram_tensor("out", (rows, 2), F32, kind="ExternalOutput")

    d_wl1 = nc.dram_tensor("w_l1", (101, 16, 128), F16, kind="ExternalInput")
    d_wl2 = nc.dram_tensor("w_l2", (128, 2, 128), F16, kind="ExternalInput")
    d_wl3 = nc.dram_tensor("w_l3", (128, 2, 16), F16, kind="ExternalInput")
    d_wr1 = nc.dram_tensor("w_rho1", (16, 2, 64), F16, kind="ExternalInput")
    d_wr2n = nc.dram_tensor("w_rho2n", (64, 64), F16, kind="ExternalInput")
    d_wr2o = nc.dram_tensor("w_rho2o", (64, 16), F16, kind="ExternalInput")
    d_wp1g = nc.dram_tensor("w_psi1g", (101, 64), F16, kind="ExternalInput")
    d_wp1 = nc.dram_tensor("w_psi1", (80, 64), F16, kind="ExternalInput")
    d_wp2 = nc.dram_tensor("w_psi2", (64, 2), F16, kind="ExternalInput")
    d_bias = nc.dram_tensor("bias", (128, 10), F32, kind="ExternalInput")
    d_iota = nc.dram_tensor("iota_d", (128, 32), F32, kind="ExternalInput")

    with tile.TileContext(nc) as tc, ExitStack() as ctx:
        singles = ctx.enter_context(tc.tile_pool(name="singles", bufs=1))
        xpool = ctx.enter_context(tc.tile_pool(name="xpool", bufs=2))
        ps_big = ctx.enter_context(tc.tile_pool(name="ps_big", bufs=5, space="PSUM"))
        ps_acc = ctx.enter_context(tc.tile_pool(name="ps_acc", bufs=2, space="PSUM"))
        ps_z = ctx.enter_context(tc.tile_pool(name="ps_z", bufs=1, space="PSUM"))

        def load(name, shape, dt, dram):
            t = singles.tile(shape, dt, tag=name, name=name + "_sb")
            nc.sync.dma_start(out=t, in_=dram[tuple(slice(None) for _ in shape)])
            return t

        w_l1 = load("w_l1", [101, 16, 128], F16, d_wl1)
        w_l2 = load("w_l2", [128, 2, 128], F16, d_wl2)
        w_l3 = load("w_l3", [128, 2, 16], F16, d_wl3)
        w_rho1 = load("w_rho1", [16, 2, 64], F16, d_wr1)
        w_rho2n = load("w_rho2n", [64, 64], F16, d_wr2n)
        w_rho2o = load("w_rho2o", [64, 16], F16, d_wr2o)
        w_psi1g = load("w_psi1g", [101, 64], F16, d_wp1g)
        w_psi1 = load("w_psi1", [80, 64], F16, d_wp1)
        w_psi2 = load("w_psi2", [64, 2], F16, d_wp2)
        bias = load("bias", [128, 10], F32, d_bias)
        iota_d = load("iota_d", [128, 32], F32, d_iota)

        psi_in = singles.tile([80, 512], F16, tag="psi_in")
        nc.vector.memset(psi_in, 0.0)

        # Warm-up: touch weights on PE; engine-local bias/iota copies so no
        # steady-state instruction spends a wait slot on constant DMAs.
        for wt in [w_l1[:, 0, :], w_l2[:, 0, :], w_l3[:, 0, :],
                   w_rho1[:, 0, :], w_rho2n, w_rho2o, w_psi1g, w_psi1,
                   w_psi2]:
            nc.tensor.ldweights(weights=wt)
        bias_a = singles.tile([128, 10], F32, tag="bias_a")
        nc.scalar.copy(bias_a, bias)
        bias_v = singles.tile([128, 10], F32, tag="bias_v")
        nc.vector.tensor_copy(out=bias_v, in_=bias)
        iota_p = singles.tile([128, 32], F32, tag="iota_p")
        nc.gpsimd.tensor_copy(out=iota_p, in_=iota_d)

        # static ping-pong work tiles
        xts = [singles.tile([128, 2048], F16, tag=f"xt{i}", name=f"xt{i}")
               for i in range(2)]
        h1s = [singles.tile([128, 2, 512], F16, tag=f"h1{i}", name=f"h1{i}")
               for i in range(3)]
        h2s = [singles.tile([128, 2, 512], F16, tag=f"h2{i}", name=f"h2{i}")
               for i in range(3)]
        rh1s = [singles.tile([128, 2, 512], F16, tag=f"rh1{i}", name=f"rh1{i}")
                for i in range(2)]
        rins = [[singles.tile([16, 512], F16, tag=f"rin{b}{i}",
                              name=f"rin{b}{i}") for i in range(2)]
                for b in range(2)]
        psihs = [singles.tile([64, 512], F16, tag=f"psih{i}", name=f"psih{i}")
                 for i in range(2)]
        d2 = singles.tile([128, 16, 32], F32, tag="d2")
        t32 = singles.tile([128, 16, 32], F32, tag="t32")
        tr = singles.tile([128, 16, 32], F32, tag="tr")
        vx = singles.tile([128, 16, 16], F32, tag="vx")
        vy = singles.tile([128, 16, 16], F32, tag="vy")
        eq = singles.tile([128, 16, 32], F32, tag="eq")
        oh = singles.tile([128, 16, 32], F32, tag="oh")

        sx_all = singles.tile([128, nchunks, 16], F32, tag="sx")
        sy_all = singles.tile([128, nchunks, 16], F32, tag="sy")
        dg_all = singles.tile([128, nchunks, 16], F32, tag="dg")
        q_all = singles.tile([128, nchunks, 16], F32, tag="q")
        zx_all = singles.tile([128, nchunks, 16], F32, tag="zx")
        zy_all = singles.tile([128, nchunks, 16], F32, tag="zy")
        out_xy = singles.tile([128, nchunks, 16, 2], F32, tag="oxy")

        x_view = x_in.rearrange("(p c s) f -> p c s f", p=128, c=nchunks)
        x16_view = x16_in.rearrange("(c r) f -> c r f", c=nchunks)
        iota_b = _bc_mid(iota_p[:, :], 16)

        def relu_copy(dst, src, bias_ap, eng):
            if eng is nc.scalar:
                nc.scalar.activation(dst, src, AF.Relu, bias=bias_ap, scale=1.0)
            else:
                nc.vector.tensor_scalar(
                    out=dst, in0=src, scalar1=bias_ap, scalar2=0.0,
                    op0=OP.add, op1=OP.max,
                )

        def tree_reduce(g_, buf, n0, op, out_ap=None):
            """In-place halving tree over the innermost dim of buf."""
            n = n0 // 2
            while n >= 1:
                dst = buf[:, :, 0:n] if (n > 1 or out_ap is None) else out_ap
                _tt(g_, dst, buf[:, :, 0:n], buf[:, :, n:2 * n], op)
                n //= 2

        def trace_branch(ch, g, xt_c, z_ps):
            xt = xt_c[0:101, g * 512:(g + 1) * 512]
            acc = ps_acc.tile([80, 512], F32, tag="acc", name="acc")
            for br in range(2):
                for p4 in range(4):
                    h1 = h1s[p4 % 3]
                    for h in range(2):
                        h1ps = ps_big.tile([128, 512], F32, tag="big",
                                           name="h1ps")
                        nc.tensor.matmul(
                            h1ps, w_l1[:, br * 8 + p4 * 2 + h, :], xt,
                            start=True, stop=True,
                        )
                        relu_copy(h1[:, h, :], h1ps,
                                  bias_a[:, 2 * br:2 * br + 1], nc.scalar)
                    h2 = h2s[p4 % 3]
                    for h in range(2):
                        h2ps = ps_big.tile([128, 512], F32, tag="big",
                                           name="h2ps")
                        nc.tensor.matmul(
                            h2ps, w_l2[:, br, :], h1[:, h, :],
                            start=True, stop=True,
                        )
                        relu_copy(h2[:, h, :], h2ps,
                                  bias_v[:, 2 * br + 1:2 * br + 2], nc.vector)
                    for h in range(2):
                        nc.tensor.matmul(
                            acc[32 * br:32 * br + 16, :],
                            w_l3[:, br, :], h2[:, h, :],
                            start=(p4 == 0 and h == 0),
                            stop=(p4 == 3 and h == 1),
                        )
            return acc

        def trace_tail(ch, g, xt_c, z_ps, acc):
            xt = xt_c[0:101, g * 512:(g + 1) * 512]
            rin_n = rins[0][g % 2]
            nc.scalar.copy(rin_n, acc[0:16, :])
            rin_o = rins[1][g % 2]
            nc.scalar.copy(rin_o, acc[32:48, :])

            rh1 = rh1s[g % 2]
            r1ps = ps_big.tile([128, 512], F32, tag="big", name="r1ps")
            nc.tensor.matmul(r1ps[0:64, :], w_rho1[:, 0, :], rin_n,
                             start=True, stop=True)
            relu_copy(rh1[0:64, 0, :], r1ps[0:64, :], bias_v[0:64, 4:5],
                      nc.vector)
            r1ps2 = ps_big.tile([128, 512], F32, tag="big", name="r1ps2")
            nc.tensor.matmul(r1ps2[0:64, :], w_rho1[:, 1, :], rin_o,
                             start=True, stop=True)
            relu_copy(rh1[0:64, 1, :], r1ps2[0:64, :], bias_v[0:64, 5:6],
                      nc.vector)

            psi_ps = ps_acc.tile([80, 512], F32, tag="acc", name="psi_ps")
            nc.tensor.matmul(psi_ps[0:64, :], w_rho2n, rh1[0:64, 0, :],
                             start=True, stop=True)
            nc.tensor.matmul(psi_ps[64:80, :], w_rho2o, rh1[0:64, 1, :],
                             start=True, stop=True)
            nc.scalar.copy(psi_in[0:80, :], psi_ps[0:80, :])

            ph_ps = ps_big.tile([128, 512], F32, tag="big", name="ph_ps")
            nc.tensor.matmul(ph_ps[0:64, :], w_psi1g, xt,
                             start=True, stop=False)
            nc.tensor.matmul(ph_ps[0:64, :], w_psi1, psi_in,
                             start=False, stop=True)
            psi_h = psihs[g % 2]
            relu_copy(psi_h, ph_ps[0:64, :], bias_a[0:64, 6:7], nc.scalar)

            for sl in range(4):
                nc.tensor.matmul(
                    z_ps[:, 4 * g + sl, :],
                    psi_h[:, sl * 128:(sl + 1) * 128], w_psi2,
                    start=True, stop=True,
                )

        def trace_apf(ch, x_ch, z_ps):
            nc.vector.tensor_copy(out=zx_all[:, ch, :], in_=z_ps[:, :, 0])
            nc.vector.tensor_copy(out=zy_all[:, ch, :], in_=z_ps[:, :, 1])

            nall = x_ch[:, :, 5:69].rearrange("p s (j c) -> p s j c", c=4)
            nx = nall[:, :, :, 0]
            ny = nall[:, :, :, 1]
            oall = x_ch[:, :, 69:101].rearrange("p s (j c) -> p s j c", c=2)
            ox = oall[:, :, :, 0]
            oy = oall[:, :, :, 1]

            g_ = nc.vector
            _tt(g_, d2[:, :, 0:16], nx, nx, OP.mult)
            _tt(g_, t32[:, :, 0:16], ny, ny, OP.mult)
            _tt(g_, d2[:, :, 0:16], d2[:, :, 0:16], t32[:, :, 0:16], OP.add)
            g_.tensor_scalar(out=t32[:, :, 0:16], in0=ox, scalar1=-0.5,
                             scalar2=0.5, op0=OP.max, op1=OP.min)
            _tt(g_, vx, ox, t32[:, :, 0:16], OP.subtract)
            g_.tensor_scalar(out=t32[:, :, 0:16], in0=oy, scalar1=-0.5,
                             scalar2=0.5, op0=OP.max, op1=OP.min)
            _tt(g_, vy, oy, t32[:, :, 0:16], OP.subtract)
            _tt(g_, d2[:, :, 16:32], vx, vx, OP.mult)
            _tt(g_, t32[:, :, 16:32], vy, vy, OP.mult)
            _tt(g_, d2[:, :, 16:32], d2[:, :, 16:32], t32[:, :, 16:32],
                OP.add)

            g_.tensor_reduce(out=tr[:, :, 0], in_=d2, axis=AX.X, op=OP.min)
            _tt(g_, eq, d2, _bc_inner(tr[:, :, 0], 32), OP.is_equal)
            _tt(g_, eq, eq, iota_b, OP.mult)
            g_.tensor_reduce(out=tr[:, :, 1], in_=eq, axis=AX.X, op=OP.max)
            _tt(g_, oh, iota_b, _bc_inner(tr[:, :, 1], 32), OP.is_equal)

            _tt(g_, t32[:, :, 0:16], oh[:, :, 0:16], nx, OP.mult)
            _tt(g_, t32[:, :, 16:32], oh[:, :, 16:32], vx, OP.mult)
            g_.tensor_reduce(out=sx_all[:, ch, :], in_=t32, axis=AX.X,
                             op=OP.add)
            _tt(g_, t32[:, :, 0:16], oh[:, :, 0:16], ny, OP.mult)
            _tt(g_, t32[:, :, 16:32], oh[:, :, 16:32], vy, OP.mult)
            g_.tensor_reduce(out=sy_all[:, ch, :], in_=t32, axis=AX.X,
                             op=OP.add)

            g_.tensor_scalar(out=dg_all[:, ch, :], in0=tr[:, :, 1],
                             scalar1=16.0, scalar2=0.15, op0=OP.is_gt,
                             op1=OP.mult)
            _tt(g_, q_all[:, ch, :], sx_all[:, ch, :], sx_all[:, ch, :],
                OP.mult)
            _tt(g_, t32[:, 0, 0:16], sy_all[:, ch, :], sy_all[:, ch, :],
                OP.mult)
            _tt(g_, q_all[:, ch, :], q_all[:, ch, :], t32[:, 0, 0:16], OP.add)

        # software pipeline: tail of stage k runs while stage k+1's branch
        # matmul stream keeps every queue fed.
        state = {}
        pending = None  # (ch, g, xt_c, z_ps, acc)
        for ch in range(nchunks):
            x_ch = xpool.tile([128, 16, 101], F32, tag="x", name="x_ch")
            nc.sync.dma_start(out=x_ch, in_=x_view[:, ch, :, :])
            xt_c = xts[ch % 2]
            nc.sync.dma_start_transpose(out=xt_c, in_=x16_view[ch, :, :])
            z_ps = ps_z.tile([128, 16, 2], F32, tag="z", name="z_ps")
            state[ch] = (x_ch, z_ps)
            for g in range(4):
                acc = trace_branch(ch, g, xt_c, z_ps)
                if pending is not None:
                    trace_tail(*pending)
                    pch, pg = pending[0], pending[1]
                    if pg == 3:
                        trace_apf(pch, *state.pop(pch))
                pending = (ch, g, xt_c, z_ps, acc)
        trace_tail(*pending)
        trace_apf(pending[0], *state.pop(pending[0]))

        # ---- epilogue ----
        normp = singles.tile([128, nchunks, 16], F32, tag="normp")
        nc.scalar.sqrt(normp, q_all)
        hb = singles.tile([128, nchunks, 16], F32, tag="hb")
        nc.vector.scalar_tensor_tensor(
            out=hb, in0=normp, scalar=0.15, in1=dg_all,
            op0=OP.subtract, op1=OP.subtract,
        )
        den = singles.tile([128, nchunks, 16], F32, tag="den")
        _tt(nc.vector, den, normp, hb, OP.mult)
        rden = singles.tile([128, nchunks, 16], F32, tag="rden")
        nc.vector.reciprocal(out=rden, in_=den)
        rsc = singles.tile([128, nchunks, 16], F32, tag="rsc")
        nc.vector.tensor_scalar_mul(out=rsc, in0=rden, scalar1=-B_GAMMA)
        bx = singles.tile([128, nchunks, 16], F32, tag="bx")
        _tt(nc.vector, bx, sx_all, rsc, OP.mult)
        by = singles.tile([128, nchunks, 16], F32, tag="by")
        _tt(nc.vector, by, sy_all, rsc, OP.mult)

        tx = singles.tile([128, nchunks, 16], F32, tag="tx")
        nc.scalar.activation(tx, zx_all, AF.Tanh, bias=bias_a[:, 7:8], scale=1.0)
        ty = singles.tile([128, nchunks, 16], F32, tag="ty")
        nc.scalar.activation(ty, zy_all, AF.Tanh, bias=bias_a[:, 8:9], scale=1.0)
        ex = singles.tile([128, nchunks, 16], F32, tag="ex")
        nc.vector.scalar_tensor_tensor(out=ex, in0=tx, scalar=2.0, in1=bx,
                                       op0=OP.mult, op1=OP.add)
        ey = singles.tile([128, nchunks, 16], F32, tag="ey")
        nc.vector.scalar_tensor_tensor(out=ey, in0=ty, scalar=2.0, in1=by,
                                       op0=OP.mult, op1=OP.add)
        fx = singles.tile([128, nchunks, 16], F32, tag="fx")
        nc.scalar.activation(fx, ex, AF.Tanh, bias=0.0, scale=1.0)
        fy = singles.tile([128, nchunks, 16], F32, tag="fy")
        nc.scalar.activation(fy, ey, AF.Tanh, bias=0.0, scale=1.0)
        nc.vector.tensor_scalar_mul(out=out_xy[:, :, :, 0], in0=fx, scalar1=2.0)
        nc.vector.tensor_scalar_mul(out=out_xy[:, :, :, 1], in0=fy, scalar1=2.0)

        out_view = out.rearrange("(p c s) t -> p c s t", p=128, c=nchunks)
        nc.sync.dma_start(out=out_view, in_=out_xy)

    _legalize_waits(nc)
    return nc


# ---------------------------------------------------------------------------
# entry point
# ---------------------------------------------------------------------------

_CACHE = {}


def _get_program(rows):
    if rows not in _CACHE:
        _CACHE[rows] = _build(rows)
    return _CACHE[rows]


def _x16_of(x, rows):
    """fp16 copy of x, 128-col padded, rows permuted per core so that the
    xbar transpose's column order (linear within a 2048-chunk) matches the
    batch-major row mapping row = p*(nchunks*16) + ch*16 + s."""
    nchunks = rows // 2048
    x16 = np.zeros((x.shape[0], 128), np.float16)
    x16[:, 0:101] = x
    p = np.arange(128)
    ch = np.arange(nchunks)
    s = np.arange(16)
    # destination index ch*2048 + s*128 + p  <-  source index p*(nchunks*16) + ch*16 + s
    src_idx = (p[None, None, :] * (nchunks * 16) + ch[:, None, None] * 16
               + s[None, :, None])  # [ch, s, p]
    perm = src_idx.reshape(-1)
    ncores = x.shape[0] // rows
    outp = np.empty_like(x16)
    for c in range(ncores):
        outp[c * rows:(c + 1) * rows] = x16[c * rows:(c + 1) * rows][perm]
    return outp


def kernel(x, phi_n, rho_n, phi_o, rho_o, psi, _rows_per_core=None,
           _core_ids=None, _trace=False):
    x = np.ascontiguousarray(np.asarray(x, dtype=np.float32))
    C = _prep_consts(phi_n, rho_n, phi_o, rho_o, psi)

    rows = _rows_per_core or (x.shape[0] // NCORES)
    core_ids = _core_ids if _core_ids is not None else list(range(NCORES))
    ncores = len(core_ids)

    nc = _get_program(rows)
    x16 = _x16_of(x, rows)

    const_map = {k: C[k] for k in
                 ["w_l1", "w_l2", "w_l3", "w_rho1", "w_rho2n", "w_rho2o",
                  "w_psi1g", "w_psi1", "w_psi2", "bias", "iota_d"]}
    in_maps = []
    for i in range(ncores):
        m = dict(const_map)
        m["x_in"] = x[i * rows: (i + 1) * rows]
        m["x16_in"] = x16[i * rows: (i + 1) * rows]
        in_maps.append(m)

    res = run_bass_kernel_spmd(nc, in_maps, core_ids=core_ids, trace=_trace)
    outp = np.concatenate([r["out"] for r in res.results], axis=0)
    if _trace:
        return outp, res
    return outp
